# revision 1
# baseline (speedup 1.0000x reference)
"""Trainium2 Bass kernel for nn_Attention_38405597560936.

GroupNorm -> qkv 1x1 conv -> 8-head self-attention over 48x48 tokens -> proj
1x1 conv -> residual.  Sharded over 8 NeuronCores: data-parallel over batch
(2) x tensor-parallel over head pairs (4).  Each core computes GN for its
batch, q/k/v for its 2 heads, the attention, and a partial proj output
(contracting only its 128 a-channels); the host sums the 4 partials per
batch and adds proj bias + v-bias contribution + residual.

Layout conventions per core (A = first head, B = second head):
  q_sb/k_sb [128, 2304] f32r: partitions 0:64 = head A dims, 64:128 = head B.
  Attention is computed transposed: ST[ki, q] = k^T q, softmax over ki
  (partition axis) via a ones-column appended to v^T in the AV matmul
  (U[64,:] = denominator).
"""
import numpy as np
import ml_dtypes
from contextlib import ExitStack, nullcontext

import concourse.bass as bass
import concourse.tile as tile
from concourse import bacc, mybir
from concourse import bass_utils

F32 = mybir.dt.float32
F32R = mybir.dt.float32r
BF16 = mybir.dt.bfloat16
MMDT = F32R            # matmul pipeline dtype: F32R (accurate, ~1.1e-4) or BF16 (~9% faster, ~4.5e-4)
AF = mybir.ActivationFunctionType
ALU = mybir.AluOpType

B, C, H, W = 2, 512, 48, 48
N = H * W                      # 2304 tokens
HEADS, D = 8, 64
GROUPS = 32                    # 16 channels per group
EPS = 1e-5
SCALE = 1.0 / 8.0              # 1/sqrt(64)
NCORES = 8
CT = C // 128                  # 4 channel tiles
NT = N // 128                  # 18 token tiles
CHUNKS = [(0, 512), (512, 512), (1024, 512), (1536, 512), (2048, 256)]

_CACHE: dict = {}


PRO = 8          # QK/exp software-prologue depth per chunk (default)


def _build(phases="abc", repeat=None, warm=True, pro=None):
    nc = bacc.Bacc("TRN2", debug=False, num_devices=NCORES)

    x32 = nc.dram_tensor("x32", [C // 2, N], F32, kind="ExternalInput").ap()
    xbf = nc.dram_tensor("xbf", [C // 2, N], BF16, kind="ExternalInput").ap()
    # packed constants: fpk = [wq(512) | wk(512) | wv(512) | ident(128) | ones(1)]
    fpk = nc.dram_tensor("fpk", [128, 1701], MMDT, kind="ExternalInput").ap()
    wp = nc.dram_tensor("wp", [64, 1024], MMDT, kind="ExternalInput").ap()
    # cpk = [ind(128) | gnsc(4) | gnbi(4) | bq(1) | bk(1)]
    cpk = nc.dram_tensor("cpk", [128, 138], F32, kind="ExternalInput").ap()
    indT = nc.dram_tensor("indT", [32, 512], F32, kind="ExternalInput").ap()

    out = nc.dram_tensor("out", [C, N], F32, kind="ExternalOutput").ap()

    PRO = pro if pro is not None else globals()["PRO"]
    with tile.TileContext(nc) as tc, ExitStack() as ctx:
        pers = ctx.enter_context(tc.tile_pool(name="pers", bufs=1))
        # one shared PSUM pool for all phases: exactly 8 banks
        #   qk (2 slots x 2 banks) | tr 1 | ua 1 | ub 1 | pp 1
        ps = ctx.enter_context(tc.tile_pool(name="ps", bufs=1, space="PSUM"))
        work = ctx.enter_context(tc.tile_pool(name="work", bufs=1))
        xp = ctx.enter_context(tc.tile_pool(name="xp", bufs=4))
        att = ctx.enter_context(tc.tile_pool(name="att", bufs=3))
        nrm = ctx.enter_context(tc.tile_pool(name="nrm", bufs=1))

        fpk_sb = pers.tile([128, 1701], MMDT)
        nc.gpsimd.dma_start(fpk_sb, fpk)
        wp_sb = pers.tile([64, 1024], MMDT)
        nc.gpsimd.dma_start(wp_sb, wp)
        cpk_sb = pers.tile([128, 138], F32)
        nc.gpsimd.dma_start(cpk_sb, cpk)
        indT_sb = pers.tile([32, 512], F32)
        nc.gpsimd.dma_start(indT_sb, indT)
        wq_sb = fpk_sb[:, 0:512]
        wk_sb = fpk_sb[:, 512:1024]
        wv_sb = fpk_sb[:, 1024:1536]
        ident = fpk_sb[:, 1536:1664]
        ones_col = fpk_sb[:, 1664:1665]
        wpa_sb = wp_sb[:, 0:512]
        wpb_sb = wp_sb[:, 512:1024]
        ind_sb = cpk_sb[:, 0:128]
        gnsc_sb = cpk_sb[:, 128:132]
        gnbi_sb = cpk_sb[:, 132:136]
        bq_sb = cpk_sb[:, 136:137]
        bk_sb = cpk_sb[:, 137:138]

        xn_sb = pers.tile([128, CT * N], MMDT)       # normalized input, c-tile major
        q_sb = pers.tile([128, N], MMDT)
        k_sb = pers.tile([128, N], MMDT)
        v_sb = pers.tile([128, N], MMDT)
        vt_sb = pers.tile([128, NT * 130], MMDT)     # [vA|1|vB|1] per token tile
        # constant ones columns of vt (positions 64 and 129 of each tile):
        # two strided DMAs replace 36 tiny DVE copies
        vt3 = vt_sb.rearrange("p (t c) -> p t c", c=130)
        nc.sync.dma_start(vt3[:, :, 64:65], fpk[:, 1665:1683])
        nc.sync.dma_start(vt3[:, :, 129:130], fpk[:, 1683:1701])

        with nc.allow_low_precision(reason="f32r compute pipeline by design"), \
                (tc.For_i(0, repeat, 1) if repeat else nullcontext()):
            # ---------------- Phase A: GroupNorm ----------------
            if warm:
                warm_t = ps.tile([128, 512], F32, tag="qk", bufs=2)
                for _ in range(16):
                    nc.tensor.matmul(warm_t, wq_sb[:, 0:128], fpk_sb[:, 0:512],
                                     start=True, stop=True)
            eps_t = work.tile([32, 1], F32)
            nc.vector.memset(eps_t, EPS)
            x_tiles = []
            gs_ps = ps.tile([32, 2], F32, tag="u")
            for ct in range(CT):
                if ct % 2 == 0:
                    x_sb = xp.tile([128, N], F32, tag="x32", bufs=2)
                    nc.sync.dma_start(x_sb, x32[(ct // 2) * 128:(ct // 2 + 1) * 128, :])
                else:
                    x_sb = xp.tile([128, N], BF16, tag="xbf", bufs=2)
                    nc.scalar.dma_start(x_sb, xbf[(ct // 2) * 128:(ct // 2 + 1) * 128, :])
                x_tiles.append(x_sb)
                m1m2 = work.tile([128, 2], F32, tag=f"mm{ct}")
                if ct % 2 == 0:
                    # DVE path: bn_stats -> (mean, E[x^2])
                    stats = work.tile([128, 9, 6], F32, tag=f"st{ct}")
                    for i in range(9):
                        nc.vector.bn_stats(stats[:, i, :],
                                           x_sb[:, i * 256:(i + 1) * 256])
                    mv = work.tile([128, 2], F32, tag=f"mv{ct}")
                    nc.vector.bn_aggr(mv, stats)
                    nc.vector.tensor_copy(m1m2[:, 0:1], mv[:, 0:1])
                    nc.vector.tensor_scalar(m1m2[:, 1:2], mv[:, 0:1], mv[:, 0:1],
                                            mv[:, 1:2], op0=ALU.mult, op1=ALU.add)
                else:
                    # ACT path: free-dim accumulate -> (sum x, sum x^2); the
                    # group-indicator matrix carries the extra 1/2304 factor
                    # for these channel tiles.
                    scr = work.tile([128, N], BF16, tag="scr")
                    nc.scalar.activation(scr, x_sb, AF.Identity,
                                         accum_out=m1m2[:, 0:1])
                    scr2 = work.tile([128, N], BF16, tag="scr")
                    nc.scalar.activation(scr2, x_sb, AF.Square,
                                         accum_out=m1m2[:, 1:2])
                nc.tensor.matmul(gs_ps, ind_sb[:, ct * 32:(ct + 1) * 32], m1m2,
                                 start=(ct == 0), stop=(ct == CT - 1))

            gs_sb = work.tile([32, 2], F32)
            nc.vector.tensor_copy(gs_sb, gs_ps)
            mu2 = work.tile([32, 1], F32)
            nc.vector.tensor_tensor(mu2, gs_sb[:, 0:1], gs_sb[:, 0:1], op=ALU.mult)
            gvar = work.tile([32, 1], F32)
            nc.vector.tensor_tensor(gvar, gs_sb[:, 1:2], mu2, op=ALU.subtract)
            # rstd = exp(-0.5 * ln(var + eps))
            lnv = work.tile([32, 1], F32)
            nc.scalar.activation(lnv, gvar, AF.Ln, bias=eps_t)
            grs = work.tile([32, 2], F32)
            nc.vector.tensor_copy(grs[:, 0:1], gs_sb[:, 0:1])
            nc.scalar.activation(grs[:, 1:2], lnv, AF.Exp, scale=-0.5)

            for ct in range(CT):
                chs_ps = ps.tile([128, 2], F32, tag=("u", "pp")[ct % 2])
                nc.tensor.matmul(chs_ps, indT_sb[:, ct * 128:(ct + 1) * 128], grs,
                                 start=True, stop=True)
                chs = work.tile([128, 2], F32, tag=f"ch{ct}")
                nc.vector.tensor_copy(chs, chs_ps)
                sc = work.tile([128, 1], F32, tag=f"sc{ct}")
                nc.vector.tensor_tensor(sc, chs[:, 1:2], gnsc_sb[:, ct:ct + 1],
                                        op=ALU.mult)
                bi = work.tile([128, 1], F32, tag=f"bi{ct}")
                nc.vector.tensor_tensor(bi, chs[:, 0:1], sc, op=ALU.mult)
                nc.vector.tensor_tensor(bi, gnbi_sb[:, ct:ct + 1], bi,
                                        op=ALU.subtract)
                if ct % 2 == 1:
                    nc.scalar.activation(xn_sb[:, ct * N:ct * N + N], x_tiles[ct],
                                         AF.Identity, bias=bi, scale=sc)
                else:
                    nc.vector.tensor_scalar(xn_sb[:, ct * N:ct * N + N],
                                            x_tiles[ct], sc, bi,
                                            op0=ALU.mult, op1=ALU.add)

            if phases == "a":
                for ct in range(CT):
                    nc.sync.dma_start(out[ct * 128:(ct + 1) * 128, :],
                                      xn_sb[:, ct * N:ct * N + N].bitcast(F32))
            # ------------- helpers for fused phases B + C -------------
            def qk_exp(c0, cw, t):
                # head B's QK output lives at column offset 512 so the two
                # concurrent row-packed matmuls never share (or span) a PSUM
                # bank — same-bank concurrent PE writes fault on HW.
                qk_ps = ps.tile([128, 1024], F32, tag="qk", bufs=2, name=f"qk{t}")
                nc.tensor.matmul(qk_ps[:, 0:cw],
                                 k_sb[0:64, t * 128:(t + 1) * 128],
                                 q_sb[0:64, c0:c0 + cw], start=True, stop=True)
                nc.tensor.matmul(qk_ps[:, 512:512 + cw],
                                 k_sb[64:128, t * 128:(t + 1) * 128],
                                 q_sb[64:128, c0:c0 + cw], start=True, stop=True)
                e_sb = att.tile([128, 1024], MMDT, tag="e", bufs=PRO + 2,
                                name=f"e{t}")
                if cw == 512:
                    nc.scalar.activation(e_sb, qk_ps, AF.Exp, scale=SCALE)
                else:
                    nc.scalar.activation(e_sb[:, 0:cw], qk_ps[:, 0:cw],
                                         AF.Exp, scale=SCALE)
                    nc.scalar.activation(e_sb[:, 512:512 + cw],
                                         qk_ps[:, 512:512 + cw],
                                         AF.Exp, scale=SCALE)
                return e_sb

            def av(u, e_sb, cw, t):
                st, sp = (t == 0), (t == NT - 1)
                o = t * 130
                nc.tensor.matmul(u[:, 0:cw], vt_sb[:, o:o + 65], e_sb[:, 0:cw],
                                 start=st, stop=sp)
                nc.tensor.matmul(u[:, 512:512 + cw], vt_sb[:, o + 65:o + 130],
                                 e_sb[:, 512:512 + cw], start=st, stop=sp)

            def norm(u, cw, ci):
                # a = U[0:64] / U[64]; den row copied straight from PSUM
                # partition 64 down to partition 0 (verified DVE shift)
                dn = nrm.tile([1, 1024], F32, tag="dn", name=f"dn{ci}")
                if cw == 512:
                    nc.vector.tensor_copy(dn, u[64:65, :])
                else:
                    nc.vector.tensor_copy(dn[:, 0:cw], u[64:65, 0:cw])
                    nc.vector.tensor_copy(dn[:, 512:512 + cw],
                                          u[64:65, 512:512 + cw])
                rc = nrm.tile([1, 1024], F32, tag="rc", name=f"rc{ci}")
                if cw == 512:
                    nc.vector.reciprocal(rc, dn)
                else:
                    nc.vector.reciprocal(rc[:, 0:cw], dn[:, 0:cw])
                    nc.vector.reciprocal(rc[:, 512:512 + cw], dn[:, 512:512 + cw])
                bc = nrm.tile([64, 1024], F32, tag="bc", name=f"bc{ci}")
                nc.gpsimd.partition_broadcast(bc[:, 0:cw], rc[:, 0:cw], channels=64)
                nc.gpsimd.partition_broadcast(bc[:, 512:512 + cw],
                                              rc[:, 512:512 + cw], channels=64)
                a_t = nrm.tile([64, 1024], MMDT, tag="at", name=f"at{ci}")
                if cw == 512:
                    nc.vector.tensor_tensor(a_t, u[0:64, :], bc, op=ALU.mult)
                else:
                    nc.vector.tensor_tensor(a_t[:, 0:cw], u[0:64, 0:cw],
                                            bc[:, 0:cw], op=ALU.mult)
                    nc.vector.tensor_tensor(a_t[:, 512:512 + cw],
                                            u[0:64, 512:512 + cw],
                                            bc[:, 512:512 + cw], op=ALU.mult)
                return a_t

            def proj(a_t, c0, cw, ci, tags=("pp", "pp", "pp", "pp")):
                for mt in range(4):
                    p_ps = ps.tile([128, cw], F32, tag=tags[mt], bufs=2 if tags[mt] == "qk" else None,
                                   padded_shape=[128, 512] if tags[mt] != "qk" else [128, 1024],
                                   name=f"pp{ci}_{mt}")
                    nc.tensor.matmul(p_ps, wpa_sb[:, mt * 128:(mt + 1) * 128],
                                     a_t[:, 0:cw], start=True, stop=False)
                    nc.tensor.matmul(p_ps, wpb_sb[:, mt * 128:(mt + 1) * 128],
                                     a_t[:, 512:512 + cw], start=False, stop=True)
                    o_sb = att.tile([128, cw], F32, tag="o", bufs=4,
                                    padded_shape=[128, 512], name=f"o{ci}_{mt}")
                    nc.vector.tensor_copy(o_sb, p_ps)
                    nc.sync.dma_start(out[mt * 128:(mt + 1) * 128, c0:c0 + cw], o_sb)

            if phases != "a":
              # ------- Phase B fused with attention chunk 0 -------
              # k for all chunks first so attention chunk 0 can stream its full
              # t-loop; remaining q/v chunks are interleaved into that loop.
              def k_chunk(ci):
                  c0, cw = CHUNKS[ci]
                  k_ps = ps.tile([128, cw], F32, tag="qk", bufs=2,
                                 padded_shape=[128, 1024], name=f"kk{ci}")
                  for ct in range(CT):
                      nc.tensor.matmul(k_ps, wk_sb[:, ct * 128:(ct + 1) * 128],
                                       xn_sb[:, ct * N + c0:ct * N + c0 + cw],
                                       start=(ct == 0), stop=(ct == CT - 1))
                  nc.vector.tensor_scalar(k_sb[:, c0:c0 + cw], k_ps, bk_sb, None,
                                          op0=ALU.add)

              def q_chunk(ci):
                  c0, cw = CHUNKS[ci]
                  q_ps = ps.tile([128, cw], F32, tag="qk", bufs=2,
                                 padded_shape=[128, 1024], name=f"q{ci}")
                  for ct in range(CT):
                      nc.tensor.matmul(q_ps, wq_sb[:, ct * 128:(ct + 1) * 128],
                                       xn_sb[:, ct * N + c0:ct * N + c0 + cw],
                                       start=(ct == 0), stop=(ct == CT - 1))
                  nc.vector.tensor_scalar(q_sb[:, c0:c0 + cw], q_ps, bq_sb, None,
                                          op0=ALU.add)

              def v_chunk(ci):
                  c0, cw = CHUNKS[ci]
                  v_ps = ps.tile([128, cw], F32, tag="tr",
                                 padded_shape=[128, 512], name=f"v{ci}")
                  for ct in range(CT):
                      nc.tensor.matmul(v_ps, wv_sb[:, ct * 128:(ct + 1) * 128],
                                       xn_sb[:, ct * N + c0:ct * N + c0 + cw],
                                       start=(ct == 0), stop=(ct == CT - 1))
                  nc.vector.tensor_copy(v_sb[:, c0:c0 + cw], v_ps)
                  for t in range(c0 // 128, (c0 + cw) // 128):
                      tr_ps = ps.tile([128, 128], MMDT, tag="tr", name=f"tr{t}")
                      nc.tensor.transpose(tr_ps, v_sb[:, t * 128:(t + 1) * 128],
                                          ident)
                      o = t * 130
                      nc.vector.tensor_copy(vt_sb[:, o:o + 64], tr_ps[:, 0:64])
                      nc.vector.tensor_copy(vt_sb[:, o + 64:o + 65], ones_col)
                      nc.vector.tensor_copy(vt_sb[:, o + 65:o + 129],
                                            tr_ps[:, 64:128])
                      nc.vector.tensor_copy(vt_sb[:, o + 129:o + 130], ones_col)

              for ci in range(len(CHUNKS)):
                  k_chunk(ci)
              q_chunk(0)
              v_chunk(0)

              cA0, cwA0 = CHUNKS[0]
              u0 = ps.tile([65, 1024], F32, tag="u")
              for t in range(NT):
                  if "c" in phases:
                      e_sb = qk_exp(cA0, cwA0, t)
                      av(u0, e_sb, cwA0, t)
                  if t % 4 == 0 and t // 4 + 1 < len(CHUNKS):
                      v_chunk(t // 4 + 1)
                  if t == 2:
                      q_chunk(1)

              # ------- attention chunks 1..4, software-pipelined -------
              prev = (u0, cA0, cwA0, 0)
              chunk_list = range(1, len(CHUNKS)) if "c" in phases else []
              for ci in chunk_list:
                  c0, cw = CHUNKS[ci]
                  u = ps.tile([65, 1024], F32, tag="u", name=f"u{ci}")
                  es = {t: qk_exp(c0, cw, t) for t in range(PRO)}
                  # previous chunk's normalize + proj land here: their PE work
                  # (proj matmuls) sits behind the prologue in the PE FIFO, so
                  # the normalize chain latency overlaps QK/exp instead of
                  # stalling the scalar engine.
                  pu, pc0, pcw, pci = prev
                  pa_t = norm(pu, pcw, pci)
                  for t in range(NT):
                      av(u, es.pop(t), cw, t)
                      if t + PRO < NT:
                          es[t + PRO] = qk_exp(c0, cw, t + PRO)
                      if t == 1 and ci + 1 < len(CHUNKS):
                          q_chunk(ci + 1)
                      if t == 3:
                          # prev chunk's proj: deferred a few steady steps so
                          # its normalize chain finishes before PE reaches it
                          proj(pa_t, pc0, pcw, pci)
                  prev = (u, c0, cw, ci)

              if "c" in phases:
                  pu, pc0, pcw, pci = prev
                  pa_t = norm(pu, pcw, pci)
                  proj(pa_t, pc0, pcw, pci, tags=("qk", "pp", "qk", "pp"))

    nc.compile()
    return nc


def _prep_core_inputs(core, xf, gn_w, gn_b, qkv_w, qkv_b, proj_w):
    """Per-core input dict. core -> (batch, head pair)."""
    b = core // 4
    hA, hB = 2 * (core % 4), 2 * (core % 4) + 1
    heads = [hA] * 64 + [hB] * 64
    dims = list(range(64)) + list(range(64))
    q_rows = np.array([h * 192 + d * 3 + 0 for h, d in zip(heads, dims)])
    k_rows = q_rows + 1
    v_rows = q_rows + 2

    # fpk: [wq(512) | wk(512) | wv(512) | ident(128) | ones(1)], c-tile major cols
    def wtiles(rows):
        # [512, 128] -> [128 partitions, 4*128 cols] c-tile major
        m = qkv_w[rows, :].T.reshape(CT, 128, 128)        # [ct][c_in, out]
        return np.concatenate([m[ct] for ct in range(CT)], axis=1)

    fpk_m = np.concatenate(
        [wtiles(q_rows), wtiles(k_rows), wtiles(v_rows),
         np.eye(128, dtype=np.float32), np.ones((128, 37), np.float32)], axis=1)

    wp_m = np.concatenate([proj_w[:, hA * 64:(hA + 1) * 64].T,
                           proj_w[:, hB * 64:(hB + 1) * 64].T], axis=1)

    ch = np.arange(C)
    grp = ch // 16
    ind_m = np.zeros((C, 32), np.float32)
    ind_m[ch, grp] = 1.0 / 16.0
    ind_m[128:256, :] /= float(N)   # ACT-path tiles (ct 1,3) provide raw sums
    ind_m[384:512, :] /= float(N)

    ind_cols = np.concatenate(
        [ind_m.reshape(CT, 128, 32)[ct] for ct in range(CT)], axis=1)  # [128, 128]
    indT_m = np.zeros((32, C), np.float32)
    indT_m[grp, ch] = 1.0
    indT_cols = np.concatenate(
        [indT_m.reshape(32, CT, 128)[:, ct, :] for ct in range(CT)], axis=1)

    cpk_m = np.concatenate(
        [ind_cols,
         gn_w.reshape(CT, 128).T, gn_b.reshape(CT, 128).T,
         qkv_b[q_rows].reshape(128, 1), qkv_b[k_rows].reshape(128, 1)], axis=1)

    mmnp = ml_dtypes.bfloat16 if MMDT == BF16 else np.float32
    return {
        "x32": np.ascontiguousarray(np.concatenate([xf[b][0:128], xf[b][256:384]]),
                                    np.float32),
        "xbf": np.ascontiguousarray(np.concatenate([xf[b][128:256], xf[b][384:512]])).astype(ml_dtypes.bfloat16),
        "fpk": np.ascontiguousarray(fpk_m).astype(mmnp),
        "wp": np.ascontiguousarray(wp_m).astype(mmnp),
        "cpk": np.ascontiguousarray(cpk_m, np.float32),
        "indT": np.ascontiguousarray(indT_cols, np.float32),
    }


last_result = None  # BassKernelResults of the most recent run (for profiling)


def kernel(x, gn_w, gn_b, qkv_w, qkv_b, proj_w, proj_b, *, trace=False):
    x = np.asarray(x, np.float32)
    gn_w = np.asarray(gn_w, np.float32)
    gn_b = np.asarray(gn_b, np.float32)
    qkv_w = np.asarray(qkv_w, np.float32)
    qkv_b = np.asarray(qkv_b, np.float32)
    proj_w = np.asarray(proj_w, np.float32)
    proj_b = np.asarray(proj_b, np.float32)

    if "nc" not in _CACHE:
        _CACHE["nc"] = _build()
    nc = _CACHE["nc"]

    xf = x.reshape(B, C, N)
    in_maps = [_prep_core_inputs(c, xf, gn_w, gn_b, qkv_w, qkv_b, proj_w)
               for c in range(NCORES)]

    res = bass_utils.run_bass_kernel_spmd(nc, in_maps, core_ids=list(range(NCORES)),
                                          trace=trace)
    global last_result
    last_result = res

    # v-bias folds to a constant per-channel vector through softmax + proj
    bv = qkv_b[np.array([h * 192 + d * 3 + 2 for h in range(HEADS) for d in range(D)])]
    cv = proj_w @ bv + proj_b                                  # [C]

    outp = np.zeros((B, C, N), np.float32)
    for core in range(NCORES):
        outp[core // 4] += res.results[core]["out"]
    outp += cv[None, :, None]
    outp += xf
    return outp.reshape(B, C, H, W)



# revision 2
# speedup vs baseline: 7.3575x; 7.3575x over previous
"""Trainium2 Bass kernel for nn_Attention_38405597560936.

GroupNorm -> qkv 1x1 conv -> 8-head self-attention over 48x48 tokens -> proj
1x1 conv -> residual.  Sharded over 8 NeuronCores: data-parallel over batch
(2) x tensor-parallel over head pairs (4).  Each core computes GN for its
batch, q/k/v for its 2 heads, the attention, and a partial proj output
(contracting only its 128 a-channels); the host sums the 4 partials per
batch and adds proj bias + v-bias contribution + residual.

Layout conventions per core (A = first head, B = second head):
  q_sb/k_sb [128, 2304] f32r: partitions 0:64 = head A dims, 64:128 = head B.
  Attention is computed transposed: ST[ki, q] = k^T q, softmax over ki
  (partition axis) via a ones-column appended to v^T in the AV matmul
  (U[64,:] = denominator).
"""
import numpy as np
import ml_dtypes
from contextlib import ExitStack, nullcontext

import concourse.bass as bass
import concourse.tile as tile
from concourse import bacc, mybir
from concourse import bass_utils

F32 = mybir.dt.float32
F32R = mybir.dt.float32r
BF16 = mybir.dt.bfloat16
MMDT = BF16            # matmul pipeline dtype: F32R (accurate, ~1.1e-4) or BF16 (~9% faster, ~4.5e-4)
AF = mybir.ActivationFunctionType
ALU = mybir.AluOpType

B, C, H, W = 2, 512, 48, 48
N = H * W                      # 2304 tokens
HEADS, D = 8, 64
GROUPS = 32                    # 16 channels per group
EPS = 1e-5
SCALE = 1.0 / 8.0              # 1/sqrt(64)
NCORES = 8
CT = C // 128                  # 4 channel tiles
NT = N // 128                  # 18 token tiles
CHUNKS = [(0, 512), (512, 512), (1024, 512), (1536, 512), (2048, 256)]

_CACHE: dict = {}


PRO = 8          # QK/exp software-prologue depth per chunk (default)


def _build(phases="abc", repeat=None, warm=True, pro=None):
    nc = bacc.Bacc("TRN2", debug=False, num_devices=NCORES)

    x32 = nc.dram_tensor("x32", [C // 2, N], F32, kind="ExternalInput").ap()
    xbf = nc.dram_tensor("xbf", [C // 2, N], BF16, kind="ExternalInput").ap()
    # packed constants: fpk = [wq(512) | wk(512) | wv(512) | ident(128) | ones(1)]
    fpk = nc.dram_tensor("fpk", [128, 1701], MMDT, kind="ExternalInput").ap()
    wp = nc.dram_tensor("wp", [64, 1024], MMDT, kind="ExternalInput").ap()
    # cpk = [ind(128) | gnsc(4) | gnbi(4) | bq(1) | bk(1)]
    cpk = nc.dram_tensor("cpk", [128, 138], F32, kind="ExternalInput").ap()
    indT = nc.dram_tensor("indT", [32, 512], F32, kind="ExternalInput").ap()

    out = nc.dram_tensor("out", [C, N], F32, kind="ExternalOutput").ap()

    PRO = pro if pro is not None else globals()["PRO"]
    with tile.TileContext(nc) as tc, ExitStack() as ctx:
        pers = ctx.enter_context(tc.tile_pool(name="pers", bufs=1))
        # one shared PSUM pool for all phases: exactly 8 banks
        #   qk (2 slots x 2 banks) | tr 1 | ua 1 | ub 1 | pp 1
        ps = ctx.enter_context(tc.tile_pool(name="ps", bufs=1, space="PSUM"))
        work = ctx.enter_context(tc.tile_pool(name="work", bufs=1))
        xp = ctx.enter_context(tc.tile_pool(name="xp", bufs=4))
        att = ctx.enter_context(tc.tile_pool(name="att", bufs=3))
        nrm = ctx.enter_context(tc.tile_pool(name="nrm", bufs=1))

        fpk_sb = pers.tile([128, 1701], MMDT)
        nc.gpsimd.dma_start(fpk_sb, fpk)
        wp_sb = pers.tile([64, 1024], MMDT)
        nc.gpsimd.dma_start(wp_sb, wp)
        cpk_sb = pers.tile([128, 138], F32)
        nc.gpsimd.dma_start(cpk_sb, cpk)
        indT_sb = pers.tile([32, 512], F32)
        nc.gpsimd.dma_start(indT_sb, indT)
        wq_sb = fpk_sb[:, 0:512]
        wk_sb = fpk_sb[:, 512:1024]
        wv_sb = fpk_sb[:, 1024:1536]
        ident = fpk_sb[:, 1536:1664]
        ones_col = fpk_sb[:, 1664:1665]
        wpa_sb = wp_sb[:, 0:512]
        wpb_sb = wp_sb[:, 512:1024]
        ind_sb = cpk_sb[:, 0:128]
        gnsc_sb = cpk_sb[:, 128:132]
        gnbi_sb = cpk_sb[:, 132:136]
        bq_sb = cpk_sb[:, 136:137]
        bk_sb = cpk_sb[:, 137:138]

        xn_sb = pers.tile([128, CT * N], MMDT)       # normalized input, c-tile major
        q_sb = pers.tile([128, N], MMDT)
        k_sb = pers.tile([128, N], MMDT)
        v_sb = pers.tile([128, N], MMDT)
        vt_sb = pers.tile([128, NT * 130], MMDT)     # [vA|1|vB|1] per token tile
        # constant ones columns of vt (positions 64 and 129 of each tile):
        # two strided DMAs replace 36 tiny DVE copies
        vt3 = vt_sb.rearrange("p (t c) -> p t c", c=130)
        nc.sync.dma_start(vt3[:, :, 64:65], fpk[:, 1665:1683])
        nc.sync.dma_start(vt3[:, :, 129:130], fpk[:, 1683:1701])

        with nc.allow_low_precision(reason="f32r compute pipeline by design"), \
                (tc.For_i(0, repeat, 1) if repeat else nullcontext()):
            # ---------------- Phase A: GroupNorm ----------------
            if warm:
                warm_t = ps.tile([128, 512], F32, tag="qk", bufs=2)
                for _ in range(16):
                    nc.tensor.matmul(warm_t, wq_sb[:, 0:128], fpk_sb[:, 0:512],
                                     start=True, stop=True)
            eps_t = work.tile([32, 1], F32)
            nc.vector.memset(eps_t, EPS)
            x_tiles = []
            gs_ps = ps.tile([32, 2], F32, tag="u")
            for ct in range(CT):
                if ct % 2 == 0:
                    x_sb = xp.tile([128, N], F32, tag="x32", bufs=2)
                    nc.sync.dma_start(x_sb, x32[(ct // 2) * 128:(ct // 2 + 1) * 128, :])
                else:
                    x_sb = xp.tile([128, N], BF16, tag="xbf", bufs=2)
                    nc.scalar.dma_start(x_sb, xbf[(ct // 2) * 128:(ct // 2 + 1) * 128, :])
                x_tiles.append(x_sb)
                m1m2 = work.tile([128, 2], F32, tag=f"mm{ct}")
                if ct % 2 == 0:
                    # DVE path: bn_stats -> (mean, E[x^2])
                    stats = work.tile([128, 9, 6], F32, tag=f"st{ct}")
                    for i in range(9):
                        nc.vector.bn_stats(stats[:, i, :],
                                           x_sb[:, i * 256:(i + 1) * 256])
                    mv = work.tile([128, 2], F32, tag=f"mv{ct}")
                    nc.vector.bn_aggr(mv, stats)
                    nc.vector.tensor_copy(m1m2[:, 0:1], mv[:, 0:1])
                    nc.vector.tensor_scalar(m1m2[:, 1:2], mv[:, 0:1], mv[:, 0:1],
                                            mv[:, 1:2], op0=ALU.mult, op1=ALU.add)
                else:
                    # ACT path: free-dim accumulate -> (sum x, sum x^2); the
                    # group-indicator matrix carries the extra 1/2304 factor
                    # for these channel tiles.
                    scr = work.tile([128, N], BF16, tag="scr")
                    nc.scalar.activation(scr, x_sb, AF.Identity,
                                         accum_out=m1m2[:, 0:1])
                    scr2 = work.tile([128, N], BF16, tag="scr")
                    nc.scalar.activation(scr2, x_sb, AF.Square,
                                         accum_out=m1m2[:, 1:2])
                nc.tensor.matmul(gs_ps, ind_sb[:, ct * 32:(ct + 1) * 32], m1m2,
                                 start=(ct == 0), stop=(ct == CT - 1))

            gs_sb = work.tile([32, 2], F32)
            nc.vector.tensor_copy(gs_sb, gs_ps)
            mu2 = work.tile([32, 1], F32)
            nc.vector.tensor_tensor(mu2, gs_sb[:, 0:1], gs_sb[:, 0:1], op=ALU.mult)
            gvar = work.tile([32, 1], F32)
            nc.vector.tensor_tensor(gvar, gs_sb[:, 1:2], mu2, op=ALU.subtract)
            # rstd = exp(-0.5 * ln(var + eps))
            lnv = work.tile([32, 1], F32)
            nc.scalar.activation(lnv, gvar, AF.Ln, bias=eps_t)
            grs = work.tile([32, 2], F32)
            nc.vector.tensor_copy(grs[:, 0:1], gs_sb[:, 0:1])
            nc.scalar.activation(grs[:, 1:2], lnv, AF.Exp, scale=-0.5)

            for ct in range(CT):
                chs_ps = ps.tile([128, 2], F32, tag=("u", "pp")[ct % 2])
                nc.tensor.matmul(chs_ps, indT_sb[:, ct * 128:(ct + 1) * 128], grs,
                                 start=True, stop=True)
                chs = work.tile([128, 2], F32, tag=f"ch{ct}")
                nc.vector.tensor_copy(chs, chs_ps)
                sc = work.tile([128, 1], F32, tag=f"sc{ct}")
                nc.vector.tensor_tensor(sc, chs[:, 1:2], gnsc_sb[:, ct:ct + 1],
                                        op=ALU.mult)
                bi = work.tile([128, 1], F32, tag=f"bi{ct}")
                nc.vector.tensor_tensor(bi, chs[:, 0:1], sc, op=ALU.mult)
                nc.vector.tensor_tensor(bi, gnbi_sb[:, ct:ct + 1], bi,
                                        op=ALU.subtract)
                if ct % 2 == 1:
                    nc.scalar.activation(xn_sb[:, ct * N:ct * N + N], x_tiles[ct],
                                         AF.Identity, bias=bi, scale=sc)
                else:
                    nc.vector.tensor_scalar(xn_sb[:, ct * N:ct * N + N],
                                            x_tiles[ct], sc, bi,
                                            op0=ALU.mult, op1=ALU.add)

            if phases == "a":
                for ct in range(CT):
                    nc.sync.dma_start(out[ct * 128:(ct + 1) * 128, :],
                                      xn_sb[:, ct * N:ct * N + N].bitcast(F32))
            # ------------- helpers for fused phases B + C -------------
            def qk_exp(c0, cw, t):
                # head B's QK output lives at column offset 512 so the two
                # concurrent row-packed matmuls never share (or span) a PSUM
                # bank — same-bank concurrent PE writes fault on HW.
                qk_ps = ps.tile([128, 1024], F32, tag="qk", bufs=2, name=f"qk{t}")
                nc.tensor.matmul(qk_ps[:, 0:cw],
                                 k_sb[0:64, t * 128:(t + 1) * 128],
                                 q_sb[0:64, c0:c0 + cw], start=True, stop=True)
                nc.tensor.matmul(qk_ps[:, 512:512 + cw],
                                 k_sb[64:128, t * 128:(t + 1) * 128],
                                 q_sb[64:128, c0:c0 + cw], start=True, stop=True)
                e_sb = att.tile([128, 1024], MMDT, tag="e", bufs=PRO + 2,
                                name=f"e{t}")
                if cw == 512:
                    nc.scalar.activation(e_sb, qk_ps, AF.Exp, scale=SCALE)
                else:
                    nc.scalar.activation(e_sb[:, 0:cw], qk_ps[:, 0:cw],
                                         AF.Exp, scale=SCALE)
                    nc.scalar.activation(e_sb[:, 512:512 + cw],
                                         qk_ps[:, 512:512 + cw],
                                         AF.Exp, scale=SCALE)
                return e_sb

            def av(u, e_sb, cw, t):
                st, sp = (t == 0), (t == NT - 1)
                o = t * 130
                nc.tensor.matmul(u[:, 0:cw], vt_sb[:, o:o + 65], e_sb[:, 0:cw],
                                 start=st, stop=sp)
                nc.tensor.matmul(u[:, 512:512 + cw], vt_sb[:, o + 65:o + 130],
                                 e_sb[:, 512:512 + cw], start=st, stop=sp)

            def norm(u, cw, ci):
                # a = U[0:64] / U[64]; den row copied straight from PSUM
                # partition 64 down to partition 0 (verified DVE shift)
                dn = nrm.tile([1, 1024], F32, tag="dn", name=f"dn{ci}")
                if cw == 512:
                    nc.vector.tensor_copy(dn, u[64:65, :])
                else:
                    nc.vector.tensor_copy(dn[:, 0:cw], u[64:65, 0:cw])
                    nc.vector.tensor_copy(dn[:, 512:512 + cw],
                                          u[64:65, 512:512 + cw])
                rc = nrm.tile([1, 1024], F32, tag="rc", name=f"rc{ci}")
                if cw == 512:
                    nc.vector.reciprocal(rc, dn)
                else:
                    nc.vector.reciprocal(rc[:, 0:cw], dn[:, 0:cw])
                    nc.vector.reciprocal(rc[:, 512:512 + cw], dn[:, 512:512 + cw])
                bc = nrm.tile([64, 1024], F32, tag="bc", name=f"bc{ci}")
                nc.gpsimd.partition_broadcast(bc[:, 0:cw], rc[:, 0:cw], channels=64)
                nc.gpsimd.partition_broadcast(bc[:, 512:512 + cw],
                                              rc[:, 512:512 + cw], channels=64)
                a_t = nrm.tile([64, 1024], MMDT, tag="at", name=f"at{ci}")
                if cw == 512:
                    nc.vector.tensor_tensor(a_t, u[0:64, :], bc, op=ALU.mult)
                else:
                    nc.vector.tensor_tensor(a_t[:, 0:cw], u[0:64, 0:cw],
                                            bc[:, 0:cw], op=ALU.mult)
                    nc.vector.tensor_tensor(a_t[:, 512:512 + cw],
                                            u[0:64, 512:512 + cw],
                                            bc[:, 512:512 + cw], op=ALU.mult)
                return a_t

            def proj(a_t, c0, cw, ci, tags=("pp", "pp", "pp", "pp")):
                for mt in range(4):
                    p_ps = ps.tile([128, cw], F32, tag=tags[mt], bufs=2 if tags[mt] == "qk" else None,
                                   padded_shape=[128, 512] if tags[mt] != "qk" else [128, 1024],
                                   name=f"pp{ci}_{mt}")
                    nc.tensor.matmul(p_ps, wpa_sb[:, mt * 128:(mt + 1) * 128],
                                     a_t[:, 0:cw], start=True, stop=False)
                    nc.tensor.matmul(p_ps, wpb_sb[:, mt * 128:(mt + 1) * 128],
                                     a_t[:, 512:512 + cw], start=False, stop=True)
                    o_sb = att.tile([128, cw], F32, tag="o", bufs=4,
                                    padded_shape=[128, 512], name=f"o{ci}_{mt}")
                    nc.vector.tensor_copy(o_sb, p_ps)
                    nc.sync.dma_start(out[mt * 128:(mt + 1) * 128, c0:c0 + cw], o_sb)

            if phases != "a":
              # ------- Phase B fused with attention chunk 0 -------
              # k for all chunks first so attention chunk 0 can stream its full
              # t-loop; remaining q/v chunks are interleaved into that loop.
              def k_chunk(ci):
                  c0, cw = CHUNKS[ci]
                  k_ps = ps.tile([128, cw], F32, tag="qk", bufs=2,
                                 padded_shape=[128, 1024], name=f"kk{ci}")
                  for ct in range(CT):
                      nc.tensor.matmul(k_ps, wk_sb[:, ct * 128:(ct + 1) * 128],
                                       xn_sb[:, ct * N + c0:ct * N + c0 + cw],
                                       start=(ct == 0), stop=(ct == CT - 1))
                  nc.vector.tensor_scalar(k_sb[:, c0:c0 + cw], k_ps, bk_sb, None,
                                          op0=ALU.add)

              def q_chunk(ci):
                  c0, cw = CHUNKS[ci]
                  q_ps = ps.tile([128, cw], F32, tag="qk", bufs=2,
                                 padded_shape=[128, 1024], name=f"q{ci}")
                  for ct in range(CT):
                      nc.tensor.matmul(q_ps, wq_sb[:, ct * 128:(ct + 1) * 128],
                                       xn_sb[:, ct * N + c0:ct * N + c0 + cw],
                                       start=(ct == 0), stop=(ct == CT - 1))
                  nc.vector.tensor_scalar(q_sb[:, c0:c0 + cw], q_ps, bq_sb, None,
                                          op0=ALU.add)

              def v_chunk(ci):
                  c0, cw = CHUNKS[ci]
                  v_ps = ps.tile([128, cw], F32, tag="tr",
                                 padded_shape=[128, 512], name=f"v{ci}")
                  for ct in range(CT):
                      nc.tensor.matmul(v_ps, wv_sb[:, ct * 128:(ct + 1) * 128],
                                       xn_sb[:, ct * N + c0:ct * N + c0 + cw],
                                       start=(ct == 0), stop=(ct == CT - 1))
                  nc.vector.tensor_copy(v_sb[:, c0:c0 + cw], v_ps)
                  for t in range(c0 // 128, (c0 + cw) // 128):
                      tr_ps = ps.tile([128, 128], MMDT, tag="tr", name=f"tr{t}")
                      nc.tensor.transpose(tr_ps, v_sb[:, t * 128:(t + 1) * 128],
                                          ident)
                      o = t * 130
                      nc.vector.tensor_copy(vt_sb[:, o:o + 64], tr_ps[:, 0:64])
                      nc.vector.tensor_copy(vt_sb[:, o + 64:o + 65], ones_col)
                      nc.vector.tensor_copy(vt_sb[:, o + 65:o + 129],
                                            tr_ps[:, 64:128])
                      nc.vector.tensor_copy(vt_sb[:, o + 129:o + 130], ones_col)

              for ci in range(len(CHUNKS)):
                  k_chunk(ci)
              q_chunk(0)
              v_chunk(0)

              cA0, cwA0 = CHUNKS[0]
              u0 = ps.tile([65, 1024], F32, tag="u")
              for t in range(NT):
                  if "c" in phases:
                      e_sb = qk_exp(cA0, cwA0, t)
                      av(u0, e_sb, cwA0, t)
                  if t % 4 == 0 and t // 4 + 1 < len(CHUNKS):
                      v_chunk(t // 4 + 1)
                  if t == 2:
                      q_chunk(1)

              # ------- attention chunks 1..4, software-pipelined -------
              prev = (u0, cA0, cwA0, 0)
              chunk_list = range(1, len(CHUNKS)) if "c" in phases else []
              for ci in chunk_list:
                  c0, cw = CHUNKS[ci]
                  u = ps.tile([65, 1024], F32, tag="u", name=f"u{ci}")
                  es = {t: qk_exp(c0, cw, t) for t in range(PRO)}
                  # previous chunk's normalize + proj land here: their PE work
                  # (proj matmuls) sits behind the prologue in the PE FIFO, so
                  # the normalize chain latency overlaps QK/exp instead of
                  # stalling the scalar engine.
                  pu, pc0, pcw, pci = prev
                  pa_t = norm(pu, pcw, pci)
                  for t in range(NT):
                      av(u, es.pop(t), cw, t)
                      if t + PRO < NT:
                          es[t + PRO] = qk_exp(c0, cw, t + PRO)
                      if t == 1 and ci + 1 < len(CHUNKS):
                          q_chunk(ci + 1)
                      if t == 3:
                          # prev chunk's proj: deferred a few steady steps so
                          # its normalize chain finishes before PE reaches it
                          proj(pa_t, pc0, pcw, pci)
                  prev = (u, c0, cw, ci)

              if "c" in phases:
                  pu, pc0, pcw, pci = prev
                  pa_t = norm(pu, pcw, pci)
                  proj(pa_t, pc0, pcw, pci, tags=("qk", "pp", "qk", "pp"))

    nc.compile()
    return nc


def _prep_core_inputs(core, xf, gn_w, gn_b, qkv_w, qkv_b, proj_w):
    """Per-core input dict. core -> (batch, head pair)."""
    b = core // 4
    hA, hB = 2 * (core % 4), 2 * (core % 4) + 1
    heads = [hA] * 64 + [hB] * 64
    dims = list(range(64)) + list(range(64))
    q_rows = np.array([h * 192 + d * 3 + 0 for h, d in zip(heads, dims)])
    k_rows = q_rows + 1
    v_rows = q_rows + 2

    # fpk: [wq(512) | wk(512) | wv(512) | ident(128) | ones(1)], c-tile major cols
    def wtiles(rows):
        # [512, 128] -> [128 partitions, 4*128 cols] c-tile major
        m = qkv_w[rows, :].T.reshape(CT, 128, 128)        # [ct][c_in, out]
        return np.concatenate([m[ct] for ct in range(CT)], axis=1)

    fpk_m = np.concatenate(
        [wtiles(q_rows), wtiles(k_rows), wtiles(v_rows),
         np.eye(128, dtype=np.float32), np.ones((128, 37), np.float32)], axis=1)

    wp_m = np.concatenate([proj_w[:, hA * 64:(hA + 1) * 64].T,
                           proj_w[:, hB * 64:(hB + 1) * 64].T], axis=1)

    ch = np.arange(C)
    grp = ch // 16
    ind_m = np.zeros((C, 32), np.float32)
    ind_m[ch, grp] = 1.0 / 16.0
    ind_m[128:256, :] /= float(N)   # ACT-path tiles (ct 1,3) provide raw sums
    ind_m[384:512, :] /= float(N)

    ind_cols = np.concatenate(
        [ind_m.reshape(CT, 128, 32)[ct] for ct in range(CT)], axis=1)  # [128, 128]
    indT_m = np.zeros((32, C), np.float32)
    indT_m[grp, ch] = 1.0
    indT_cols = np.concatenate(
        [indT_m.reshape(32, CT, 128)[:, ct, :] for ct in range(CT)], axis=1)

    cpk_m = np.concatenate(
        [ind_cols,
         gn_w.reshape(CT, 128).T, gn_b.reshape(CT, 128).T,
         qkv_b[q_rows].reshape(128, 1), qkv_b[k_rows].reshape(128, 1)], axis=1)

    mmnp = ml_dtypes.bfloat16 if MMDT == BF16 else np.float32
    return {
        "x32": np.ascontiguousarray(np.concatenate([xf[b][0:128], xf[b][256:384]]),
                                    np.float32),
        "xbf": np.ascontiguousarray(np.concatenate([xf[b][128:256], xf[b][384:512]])).astype(ml_dtypes.bfloat16),
        "fpk": np.ascontiguousarray(fpk_m).astype(mmnp),
        "wp": np.ascontiguousarray(wp_m).astype(mmnp),
        "cpk": np.ascontiguousarray(cpk_m, np.float32),
        "indT": np.ascontiguousarray(indT_cols, np.float32),
    }


last_result = None  # BassKernelResults of the most recent run (for profiling)


def kernel(x, gn_w, gn_b, qkv_w, qkv_b, proj_w, proj_b, *, trace=False):
    x = np.asarray(x, np.float32)
    gn_w = np.asarray(gn_w, np.float32)
    gn_b = np.asarray(gn_b, np.float32)
    qkv_w = np.asarray(qkv_w, np.float32)
    qkv_b = np.asarray(qkv_b, np.float32)
    proj_w = np.asarray(proj_w, np.float32)
    proj_b = np.asarray(proj_b, np.float32)

    if "nc" not in _CACHE:
        _CACHE["nc"] = _build()
    nc = _CACHE["nc"]

    xf = x.reshape(B, C, N)
    in_maps = [_prep_core_inputs(c, xf, gn_w, gn_b, qkv_w, qkv_b, proj_w)
               for c in range(NCORES)]

    res = bass_utils.run_bass_kernel_spmd(nc, in_maps, core_ids=list(range(NCORES)),
                                          trace=trace)
    global last_result
    last_result = res

    # v-bias folds to a constant per-channel vector through softmax + proj
    bv = qkv_b[np.array([h * 192 + d * 3 + 2 for h in range(HEADS) for d in range(D)])]
    cv = proj_w @ bv + proj_b                                  # [C]

    outp = np.zeros((B, C, N), np.float32)
    for core in range(NCORES):
        outp[core // 4] += res.results[core]["out"]
    outp += cv[None, :, None]
    outp += xf
    return outp.reshape(B, C, H, W)



# revision 7
# speedup vs baseline: 7.7467x; 1.0529x over previous
"""Trainium2 Bass kernel for nn_Attention_38405597560936.

GroupNorm -> qkv 1x1 conv -> 8-head self-attention over 48x48 tokens -> proj
1x1 conv -> residual.  Sharded over 8 NeuronCores: data-parallel over batch
(2) x tensor-parallel over head pairs (4).  Each core computes GN stats for
its batch, q/k/v for its 2 heads, the attention, and a partial proj output
(contracting only its 128 a-channels); the host sums the 4 partials per
batch and adds proj bias + v-bias contribution + residual.

v2 design (all-bf16 matmul pipeline):
  - GroupNorm is folded into the qkv weights on device: per-channel
    (sc, bi) from the group stats scale the weight columns (w_eff = w*sc)
    and contribute a runtime bias (w @ bi), so the big normalized-x tensor
    never exists and qkv matmuls read raw x (bf16).
  - Attention runs in 5 units over the query axis: 4 single-head 1024-wide
    units + one 256-wide both-heads tail.  Per (unit, token-tile): one QK
    matmul (bf16, 1024-wide moving), one Exp activation, one AV matmul.
  - Softmax denominator via a ones-column appended to v^T (U[64] = den).
  - Layout: q_sb/k_sb/v_sb [128, 2304], partitions 0:64 head A, 64:128 B.
"""
import numpy as np
import ml_dtypes
from contextlib import ExitStack, nullcontext

import concourse.bass as bass
import concourse.tile as tile
from concourse import bacc, mybir
from concourse import bass_utils

F32 = mybir.dt.float32
BF16 = mybir.dt.bfloat16
MMDT = BF16
AF = mybir.ActivationFunctionType
ALU = mybir.AluOpType

B, C, H, W = 2, 512, 48, 48
N = H * W                      # 2304 tokens
HEADS, D = 8, 64
GROUPS = 32                    # 16 channels per group
EPS = 1e-5
SCALE = 1.0 / 8.0              # 1/sqrt(64)
NCORES = 8
CT = C // 128                  # 4 channel tiles
NT = N // 128                  # 18 token tiles
# query chunks (both heads per chunk; head A at cols 0:cw, B at 512:512+cw)
CHUNKS = [(0, 512), (512, 512), (1024, 512), (1536, 512), (2048, 256)]

_CACHE: dict = {}

PRO = 8          # QK/exp software-prologue depth per unit


def _build(phases="abc", repeat=None, warm=True, pro=None):
    nc = bacc.Bacc("TRN2", debug=False, num_devices=NCORES)

    # x, c-tile major: [128, 4*2304] bf16
    xbf = nc.dram_tensor("xbf", [128, CT * N], BF16, kind="ExternalInput").ap()
    # packed consts: fpk = [wq(512) | wk(512) | wv(512) | ident(128) | ones(37)]
    fpk = nc.dram_tensor("fpk", [128, 1701], MMDT, kind="ExternalInput").ap()
    wp = nc.dram_tensor("wp", [64, 1024], MMDT, kind="ExternalInput").ap()
    # cpk = [ind(128) | gnsc(4) | gnbi(4) | bq(1) | bk(1)]
    cpk = nc.dram_tensor("cpk", [128, 138], F32, kind="ExternalInput").ap()
    indT = nc.dram_tensor("indT", [32, 512], F32, kind="ExternalInput").ap()

    out = nc.dram_tensor("out", [C, N], F32, kind="ExternalOutput").ap()

    PRO = pro if pro is not None else globals()["PRO"]
    with tile.TileContext(nc) as tc, ExitStack() as ctx:
        pers = ctx.enter_context(tc.tile_pool(name="pers", bufs=1))
        # PSUM: qk (2 slots x 2 banks) | u (2 banks) | pp 1 | tr 1  = 8 banks
        ps = ctx.enter_context(tc.tile_pool(name="ps", bufs=1, space="PSUM"))
        work = ctx.enter_context(tc.tile_pool(name="work", bufs=1))
        xp = ctx.enter_context(tc.tile_pool(name="xp", bufs=2))
        att = ctx.enter_context(tc.tile_pool(name="att", bufs=3))
        nrm = ctx.enter_context(tc.tile_pool(name="nrm", bufs=1))
        wef = ctx.enter_context(tc.tile_pool(name="wef", bufs=2))

        fpk_sb = pers.tile([128, 1701], MMDT)
        nc.gpsimd.dma_start(fpk_sb, fpk)
        wp_sb = pers.tile([64, 1024], MMDT)
        nc.gpsimd.dma_start(wp_sb, wp)
        cpk_sb = pers.tile([128, 138], F32)
        nc.gpsimd.dma_start(cpk_sb, cpk)
        indT_sb = pers.tile([32, 512], F32)
        nc.gpsimd.dma_start(indT_sb, indT)
        wq_sb = fpk_sb[:, 0:512]
        wk_sb = fpk_sb[:, 512:1024]
        wv_sb = fpk_sb[:, 1024:1536]
        ident = fpk_sb[:, 1536:1664]
        wpa_sb = wp_sb[:, 0:512]
        wpb_sb = wp_sb[:, 512:1024]
        ind_sb = cpk_sb[:, 0:128]
        gnsc_sb = cpk_sb[:, 128:132]
        gnbi_sb = cpk_sb[:, 132:136]
        bq_sb = cpk_sb[:, 136:137]
        bk_sb = cpk_sb[:, 137:138]

        q_sb = pers.tile([128, N], MMDT)
        k_sb = pers.tile([128, N], MMDT)
        v_sb = pers.tile([128, N], MMDT)
        vt_sb = pers.tile([128, NT * 130], MMDT)     # [vA|1|vB|1] per token tile
        vt3 = vt_sb.rearrange("p (t c) -> p t c", c=130)
        # constant ones columns of vt (positions 64 and 129 of each tile)
        nc.sync.dma_start(vt3[:, :, 64:65], fpk[:, 1665:1683])
        nc.sync.dma_start(vt3[:, :, 129:130], fpk[:, 1683:1701])

        eps_t = pers.tile([32, 1], F32)
        nc.vector.memset(eps_t, EPS)

        if warm:
            for _ in range(16):
                warm_t = ps.tile([128, 1024], F32, tag="qk", bufs=2, name="warm")
                nc.tensor.matmul(warm_t[:, 0:512], wq_sb[:, 0:128],
                                 fpk_sb[:, 0:512], start=True, stop=True)

        with nc.allow_low_precision(reason="bf16 compute pipeline by design"), \
                (tc.For_i(0, repeat, 1) if repeat else nullcontext()):
            # ---------------- GroupNorm stats -> (sc, bi) per c-tile --------
            x_sb = xp.tile([128, CT, N], BF16, tag="x", bufs=2)
            nc.sync.dma_start(x_sb, xbf)
            gs_ps = ps.tile([32, 2], F32, tag="qk", bufs=2, name="gs",
                            padded_shape=[32, 1024])
            for ct in range(CT):
                stats = work.tile([128, 5, 6], F32, tag=f"st{ct}")
                for i in range(4):
                    nc.vector.bn_stats(stats[:, i, :],
                                       x_sb[:, ct, i * 512:(i + 1) * 512])
                nc.vector.bn_stats(stats[:, 4, :], x_sb[:, ct, 2048:2304])
                mv = work.tile([128, 2], F32, tag=f"mv{ct}")
                nc.vector.bn_aggr(mv, stats)
                m1m2 = work.tile([128, 2], F32, tag=f"mm{ct}")
                nc.vector.tensor_copy(m1m2[:, 0:1], mv[:, 0:1])
                nc.vector.tensor_scalar(m1m2[:, 1:2], mv[:, 0:1], mv[:, 0:1],
                                        mv[:, 1:2], op0=ALU.mult, op1=ALU.add)
                nc.tensor.matmul(gs_ps, ind_sb[:, ct * 32:(ct + 1) * 32], m1m2,
                                 start=(ct == 0), stop=(ct == CT - 1))

            gs_sb = work.tile([32, 2], F32)
            nc.vector.tensor_copy(gs_sb, gs_ps)
            mu2 = work.tile([32, 1], F32)
            nc.vector.tensor_tensor(mu2, gs_sb[:, 0:1], gs_sb[:, 0:1],
                                    op=ALU.mult)
            gvar = work.tile([32, 1], F32)
            nc.vector.tensor_tensor(gvar, gs_sb[:, 1:2], mu2, op=ALU.subtract)
            # rstd = exp(-0.5 * ln(var + eps))  (keeps ACT on one table set)
            lnv = work.tile([32, 1], F32)
            nc.scalar.activation(lnv, gvar, AF.Ln, bias=eps_t)
            grs = work.tile([32, 2], F32)
            nc.vector.tensor_copy(grs[:, 0:1], gs_sb[:, 0:1])
            nc.scalar.activation(grs[:, 1:2], lnv, AF.Exp, scale=-0.5)

            # per-channel (sc, bi); fold sc into the qkv weights
            weff = wef.tile([128, 1536], MMDT, tag="w")
            wq_e, wk_e, wv_e = (weff[:, 0:512], weff[:, 512:1024],
                                weff[:, 1024:1536])
            bis = []
            for ct in range(CT):
                chs_ps = ps.tile([128, 2], F32, tag="qk", bufs=2,
                                 padded_shape=[128, 1024], name=f"chs{ct}")
                nc.tensor.matmul(chs_ps, indT_sb[:, ct * 128:(ct + 1) * 128],
                                 grs, start=True, stop=True)
                chs = work.tile([128, 2], F32, tag=f"ch{ct}")
                nc.vector.tensor_copy(chs, chs_ps)
                sc = work.tile([128, 1], F32, tag=f"sc{ct}")
                nc.vector.tensor_tensor(sc, chs[:, 1:2], gnsc_sb[:, ct:ct + 1],
                                        op=ALU.mult)
                bi = work.tile([128, 1], F32, tag=f"bif{ct}")
                nc.vector.tensor_tensor(bi, chs[:, 0:1], sc, op=ALU.mult)
                bi_b = work.tile([128, 1], BF16, tag=f"bi{ct}")
                nc.vector.tensor_tensor(bi_b, gnbi_sb[:, ct:ct + 1], bi,
                                        op=ALU.subtract)
                bis.append(bi_b)
                o = ct * 128
                nc.vector.tensor_scalar(wq_e[:, o:o + 128], wq_sb[:, o:o + 128],
                                        sc, None, op0=ALU.mult)
                nc.vector.tensor_scalar(wk_e[:, o:o + 128], wk_sb[:, o:o + 128],
                                        sc, None, op0=ALU.mult)
                nc.vector.tensor_scalar(wv_e[:, o:o + 128], wv_sb[:, o:o + 128],
                                        sc, None, op0=ALU.mult)

            # runtime bias: b_eff = W @ bi (+ conv bias for q/k)
            btot = wef.tile([128, 3], F32, tag="b")
            for wi, wsb in enumerate((wq_sb, wk_sb, wv_sb)):
                be_ps = ps.tile([128, 1], F32, tag="qk", bufs=2,
                                padded_shape=[128, 1024], name=f"be{wi}")
                for ct in range(CT):
                    nc.tensor.matmul(be_ps, wsb[:, ct * 128:(ct + 1) * 128],
                                     bis[ct], start=(ct == 0),
                                     stop=(ct == CT - 1))
                if wi == 0:
                    nc.vector.tensor_scalar(btot[:, 0:1], be_ps, bq_sb, None,
                                            op0=ALU.add)
                elif wi == 1:
                    nc.vector.tensor_scalar(btot[:, 1:2], be_ps, bk_sb, None,
                                            op0=ALU.add)
                else:
                    nc.vector.tensor_copy(btot[:, 2:3], be_ps)

            # ---------------- q/k/v chunks --------------------------------
            def kq_chunk(which, ci):
                c0, cw = CHUNKS[ci]
                w_e = wk_e if which == "k" else wq_e
                dst = k_sb if which == "k" else q_sb
                bcol = 1 if which == "k" else 0
                p = ps.tile([128, cw], F32, tag="qk", bufs=2,
                            padded_shape=[128, 1024], name=f"{which}{ci}")
                for ct in range(CT):
                    nc.tensor.matmul(p, w_e[:, ct * 128:(ct + 1) * 128],
                                     x_sb[:, ct, c0:c0 + cw],
                                     start=(ct == 0), stop=(ct == CT - 1))
                nc.vector.tensor_scalar(dst[:, c0:c0 + cw], p,
                                        btot[:, bcol:bcol + 1], None,
                                        op0=ALU.add)

            def v_chunk(ci):
                c0, cw = CHUNKS[ci]
                v_ps = ps.tile([128, cw], F32, tag="tr",
                               padded_shape=[128, 512], name=f"v{ci}")
                for ct in range(CT):
                    nc.tensor.matmul(v_ps, wv_e[:, ct * 128:(ct + 1) * 128],
                                     x_sb[:, ct, c0:c0 + cw],
                                     start=(ct == 0), stop=(ct == CT - 1))
                nc.vector.tensor_scalar(v_sb[:, c0:c0 + cw], v_ps,
                                        btot[:, 2:3], None, op0=ALU.add)
                for t in range(c0 // 128, (c0 + cw) // 128):
                    tr_ps = ps.tile([128, 128], MMDT, tag="tr", name=f"tr{t}")
                    nc.tensor.transpose(tr_ps, v_sb[:, t * 128:(t + 1) * 128],
                                        ident)
                    nc.vector.tensor_copy(vt3[:, t, 0:64], tr_ps[:, 0:64])
                    nc.vector.tensor_copy(vt3[:, t, 65:129], tr_ps[:, 64:128])

            # ---------------- attention helpers ---------------------------
            def qk_exp(ci, t):
                c0, cw = CHUNKS[ci]
                # head B's QK at column offset 512 so the two concurrent
                # matmuls never share a PSUM bank
                qk_ps = ps.tile([128, 1024], F32, tag="qk", bufs=2,
                                name=f"qk{ci}_{t}")
                e_sb = att.tile([128, 1024], MMDT, tag="e", bufs=PRO + 2,
                                name=f"e{ci}_{t}")
                nc.tensor.matmul(qk_ps[:, 0:cw],
                                 k_sb[0:64, t * 128:(t + 1) * 128],
                                 q_sb[0:64, c0:c0 + cw], start=True, stop=True)
                nc.tensor.matmul(qk_ps[:, 512:512 + cw],
                                 k_sb[64:128, t * 128:(t + 1) * 128],
                                 q_sb[64:128, c0:c0 + cw], start=True,
                                 stop=True)
                if cw == 512:
                    nc.scalar.activation(e_sb, qk_ps, AF.Exp, scale=SCALE)
                else:
                    # single strided exp over both heads' blocks
                    src = qk_ps.rearrange("p (b c) -> p b c", c=512)[:, :, 0:cw]
                    dst = e_sb.rearrange("p (b c) -> p b c", c=cw)[:, 0:2, :]
                    nc.scalar.activation(dst, src, AF.Exp, scale=SCALE)
                return e_sb

            def av(u, ci, e_sb, t):
                c0, cw = CHUNKS[ci]
                st, sp = (t == 0), (t == NT - 1)
                eB = e_sb[:, 512:512 + cw] if cw == 512 else e_sb[:, cw:2 * cw]
                nc.tensor.matmul(u[:, 0:cw], vt3[:, t, 0:65],
                                 e_sb[:, 0:cw], start=st, stop=sp)
                nc.tensor.matmul(u[:, 512:512 + cw], vt3[:, t, 65:130],
                                 eB, start=st, stop=sp)

            def norm(u, ci):
                c0, cw = CHUNKS[ci]
                dn = nrm.tile([1, 1024], F32, tag="dn", name=f"dn{ci}")
                rc = nrm.tile([1, 1024], F32, tag="rc", name=f"rc{ci}")
                bc = nrm.tile([64, 1024], F32, tag="bc", name=f"bc{ci}")
                a_t = nrm.tile([64, 1024], MMDT, tag="at", bufs=2,
                               name=f"at{ci}")
                if cw == 512:
                    nc.vector.tensor_copy(dn, u[64:65, :])
                    nc.vector.reciprocal(rc, dn)
                    nc.gpsimd.partition_broadcast(bc, rc, channels=64)
                    nc.vector.tensor_tensor(a_t, u[0:64, :], bc, op=ALU.mult)
                else:
                    u3 = u.rearrange("p (b c) -> p b c", c=512)[:, :, 0:cw]
                    dn2 = dn.rearrange("p (b c) -> p b c", c=cw)[:, 0:2, :]
                    nc.vector.tensor_copy(dn2, u3[64:65])
                    nc.vector.reciprocal(rc[:, 0:2 * cw], dn[:, 0:2 * cw])
                    nc.gpsimd.partition_broadcast(bc[:, 0:2 * cw],
                                                  rc[:, 0:2 * cw], channels=64)
                    at2 = a_t.rearrange("p (b c) -> p b c", c=cw)[:, 0:2, :]
                    nc.vector.tensor_tensor(at2, u3[0:64], bc.rearrange(
                        "p (b c) -> p b c", c=cw)[:, 0:2, :], op=ALU.mult)
                return a_t

            def proj(a_t, ci, tags=("pp", "tr")):
                # a_t layout: head A at cols 0:cw, head B at cw:2cw (tail)
                # or 512:512+cw (cw=512); psum tags alternate so the WAR on
                # the o-copy is distance-2
                c0, cw = CHUNKS[ci]
                aB0 = 512 if cw == 512 else cw
                for mt in range(4):
                    tg = tags[mt % 2]
                    p_ps = ps.tile([128, cw], F32, tag=tg,
                                   bufs=2 if tg == "qk" else None,
                                   padded_shape=[128, 1024]
                                   if tg == "qk" else [128, 512],
                                   name=f"pp{ci}_{mt}")
                    nc.tensor.matmul(p_ps, wpa_sb[:, mt * 128:(mt + 1) * 128],
                                     a_t[:, 0:cw], start=True, stop=False)
                    nc.tensor.matmul(p_ps, wpb_sb[:, mt * 128:(mt + 1) * 128],
                                     a_t[:, aB0:aB0 + cw], start=False,
                                     stop=True)
                    o_sb = att.tile([128, cw], F32, tag="o", bufs=4,
                                    padded_shape=[128, 512], name=f"o{ci}_{mt}")
                    nc.vector.tensor_copy(o_sb, p_ps)
                    nc.sync.dma_start(out[mt * 128:(mt + 1) * 128,
                                          c0:c0 + cw], o_sb)

            # ---------------- schedule ------------------------------------
            # k/q chunk 0 then the first prologue QKs immediately, so ACT
            # reaches the first exp as early as possible after an iteration
            # boundary; v/k/q chunks stream in behind.
            kq_chunk("k", 0)
            kq_chunk("q", 0)

            u0 = ps.tile([65, 1024], F32, tag="u", name="u0")
            es = {t: qk_exp(0, t) for t in range(4)}
            kq_chunk("k", 1)
            es.update({t: qk_exp(0, t) for t in range(4, PRO)})
            v_chunk(0)
            v_chunk(1)
            kq_chunk("k", 2)
            prev = (u0, 0)
            # NOTE: a k/v chunk must be emitted BEFORE any qk_exp/av that
            # reads its tiles — qk_exp(t+PRO) at step t reads k tile t+PRO
            for t in range(NT):
                av(u0, 0, es.pop(t), t)
                if t + PRO < NT:
                    es[t + PRO] = qk_exp(0, t + PRO)
                if t == 0:
                    v_chunk(2)
                elif t == 2:
                    kq_chunk("k", 3)
                elif t == 4:
                    v_chunk(3)
                elif t == 6:
                    kq_chunk("k", 4)
                elif t == 8:
                    v_chunk(4)
                elif t == 12:
                    kq_chunk("q", 1)

            # chunks 1..4, software-pipelined; norm(prev) in the prologue
            # shadow, proj(prev) a few steady steps in
            for ci in range(1, len(CHUNKS)):
                u = ps.tile([65, 1024], F32, tag="u", name=f"u{ci}")
                es = {t: qk_exp(ci, t) for t in range(PRO)}
                pu, pci = prev
                pa_t = norm(pu, pci)
                for t in range(NT):
                    av(u, ci, es.pop(t), t)
                    if t + PRO < NT:
                        es[t + PRO] = qk_exp(ci, t + PRO)
                    if t == 1 and ci + 1 < len(CHUNKS):
                        kq_chunk("q", ci + 1)
                    elif t == 3:
                        proj(pa_t, pci)
                prev = (u, ci)

            pu, pci = prev
            pa_t = norm(pu, pci)
            proj(pa_t, pci, tags=("qk", "pp"))

    nc.compile()
    return nc


def _prep_core_inputs(core, xf, gn_w, gn_b, qkv_w, qkv_b, proj_w):
    """Per-core input dict. core -> (batch, head pair)."""
    b = core // 4
    hA, hB = 2 * (core % 4), 2 * (core % 4) + 1
    heads = [hA] * 64 + [hB] * 64
    dims = list(range(64)) + list(range(64))
    q_rows = np.array([h * 192 + d * 3 + 0 for h, d in zip(heads, dims)])
    k_rows = q_rows + 1
    v_rows = q_rows + 2

    # fpk: [wq(512) | wk(512) | wv(512) | ident(128) | ones(37)], c-tile major
    def wtiles(rows):
        m = qkv_w[rows, :].T.reshape(CT, 128, 128)        # [ct][c_in, out]
        return np.concatenate([m[ct] for ct in range(CT)], axis=1)

    fpk_m = np.concatenate(
        [wtiles(q_rows), wtiles(k_rows), wtiles(v_rows),
         np.eye(128, dtype=np.float32), np.ones((128, 37), np.float32)],
        axis=1)

    wp_m = np.concatenate([proj_w[:, hA * 64:(hA + 1) * 64].T,
                           proj_w[:, hB * 64:(hB + 1) * 64].T], axis=1)

    ch = np.arange(C)
    grp = ch // 16
    ind_m = np.zeros((C, 32), np.float32)
    ind_m[ch, grp] = 1.0 / 16.0

    ind_cols = np.concatenate(
        [ind_m.reshape(CT, 128, 32)[ct] for ct in range(CT)], axis=1)
    indT_m = np.zeros((32, C), np.float32)
    indT_m[grp, ch] = 1.0
    indT_cols = np.concatenate(
        [indT_m.reshape(32, CT, 128)[:, ct, :] for ct in range(CT)], axis=1)

    cpk_m = np.concatenate(
        [ind_cols,
         gn_w.reshape(CT, 128).T, gn_b.reshape(CT, 128).T,
         qkv_b[q_rows].reshape(128, 1), qkv_b[k_rows].reshape(128, 1)], axis=1)

    # x c-tile major [128, 4*2304] bf16
    x_ct = xf[b].reshape(CT, 128, N)
    x_m = np.concatenate([x_ct[ct] for ct in range(CT)], axis=1)

    mmnp = ml_dtypes.bfloat16
    return {
        "xbf": np.ascontiguousarray(x_m).astype(mmnp),
        "fpk": np.ascontiguousarray(fpk_m).astype(mmnp),
        "wp": np.ascontiguousarray(wp_m).astype(mmnp),
        "cpk": np.ascontiguousarray(cpk_m, np.float32),
        "indT": np.ascontiguousarray(indT_cols, np.float32),
    }


last_result = None  # BassKernelResults of the most recent run (for profiling)


def kernel(x, gn_w, gn_b, qkv_w, qkv_b, proj_w, proj_b, *, trace=False):
    x = np.asarray(x, np.float32)
    gn_w = np.asarray(gn_w, np.float32)
    gn_b = np.asarray(gn_b, np.float32)
    qkv_w = np.asarray(qkv_w, np.float32)
    qkv_b = np.asarray(qkv_b, np.float32)
    proj_w = np.asarray(proj_w, np.float32)
    proj_b = np.asarray(proj_b, np.float32)

    if "nc" not in _CACHE:
        _CACHE["nc"] = _build()
    nc = _CACHE["nc"]

    xf = x.reshape(B, C, N)
    in_maps = [_prep_core_inputs(c, xf, gn_w, gn_b, qkv_w, qkv_b, proj_w)
               for c in range(NCORES)]

    res = bass_utils.run_bass_kernel_spmd(nc, in_maps,
                                          core_ids=list(range(NCORES)),
                                          trace=trace)
    global last_result
    last_result = res

    # v-bias folds to a constant per-channel vector through softmax + proj
    bv = qkv_b[np.array([h * 192 + d * 3 + 2 for h in range(HEADS)
                         for d in range(D)])]
    cv = proj_w @ bv + proj_b                                  # [C]

    outp = np.zeros((B, C, N), np.float32)
    for core in range(NCORES):
        outp[core // 4] += res.results[core]["out"]
    outp += cv[None, :, None]
    outp += xf
    return outp.reshape(B, C, H, W)
